# revision 2
# baseline (speedup 1.0000x reference)
"""Trainium2 Bass kernel v2 for truncated BCH on 3D vector fields.

Math (matches the jax reference):
  out_i = l_i + r_i + 0.25 * sum_j ( D_j l_i * r_j  -  D_j r_i * l_j )
where D_j v = v[.+1] - v[.-1] along spatial axis j (circulant wrap).

Design (v2):
  - Inputs arrive as per-channel PAIR tiles (Y, 2, xh, ZP) fp16 holding
    [l_i | r_i]; every x/z diff and product is ONE instruction covering
    both sides via strided access patterns:
      * diff pair op: in0[c,x] = t[c, x+1-2c], in1[c,x] = t[c, x-1+2c]
        -> [D l | -D r]  (r-side sign folded by the reversed stencil)
      * product pair op: multiplicand AP [r_j | l_j] read from channel-j's
        pair tile with a negative stride on the pair axis.
  - Y diffs on PE with weights [0.25*DyT | -0.25*DyT] into one PSUM pair
    tile; ScalarE evacuates to fp16; DVE multiplies by [r_1 | l_1].
  - Products accumulate into a single-bank PSUM acc via 0.25I / I weight
    matmuls (0.25 pre-folded, so no final scale op is needed).
  - Linear term: s_i = l_i + r_i on DVE/Pool; the final evac is ONE fused
    tensor_add: out_fp16 = acc_psum + s (no PE inject, no scale pass).
  - Output is written fp16 (host upcasts); rel-err stays ~1e-3.
"""

import sys

sys.path.insert(0, "/opt/trn_rl_repo")

import numpy as np

import concourse.bass as bass
import concourse.bacc as bacc
import concourse.mybir as mybir
import concourse.tile as tile
from concourse.ap import AP
from concourse.bass_utils import run_bass_kernel_spmd

B, D, X, Y, Z = 2, 3, 128, 128, 128
NCORES = 8
XS = (B * X) // NCORES      # 32 output x-planes per core
XH = XS + 2                 # with 1-plane halo each side
ZP = Z + 2                  # z padded: [z127, z0..z127, z0]
KX = 4                      # x-planes per PSUM acc chunk (512 f32 = 1 bank)
CHUNKS = [2, 6, 8, 8, 4, 2, 2]  # x-planes per compute work item
DEPTH = 2                   # stage_b pipeline lag (work items)

F16 = mybir.dt.float16
F32 = mybir.dt.float32


def _make_wmats() -> np.ndarray:
    """[0.25*DyT | -0.25*DyT | I | 0.25*I] as one (Y, 4Y) fp16 matrix."""
    e = np.eye(Y, dtype=np.float32)
    dy = np.roll(e, -1, axis=0) - np.roll(e, 1, axis=0)
    dyt = dy.T
    mats = np.concatenate([0.25 * dyt, -0.25 * dyt, e, 0.25 * e], axis=1)
    return mats.astype(np.float16)


# engine assignment per op kind; value is a function(item_idx, ch) -> engine
# name "v" (DVE) or "g" (Pool).  "route" selects the linear-term path per
# work item: "c" = s-add + fused evac-add (DVE/Pool), "sigma" = inject l,r
# via I-weight matmuls on PE + plain ScalarE copy evac.  Tuned by sweeping.
ENG = {
    "dX": lambda it, i: "v",
    "pX": lambda it, i: "v",
    "dZ": lambda it, i: "g",
    "pZ": lambda it, i: "g",
    "pY": lambda it, i: "v",
    "s":  lambda it, i: "v",
    "evac": lambda it, i: "g",
    "route": lambda it, i: "sigma" if (i == 2 or (i == 1 and it == 0)) else "c",
    "odma": lambda it, i: "s",
}


def build_nc(xs: int = XS) -> bass.Bass:
    xh = xs + 2
    nc = bacc.Bacc(None)

    in_h = [nc.declare_dram_parameter(f"ch{i}", [Y, xh, 2, ZP], F16,
                                      isOutput=False) for i in range(D)]
    w_h = nc.declare_dram_parameter("wmats", [Y, 4 * Y], F16, isOutput=False)
    out_h = nc.declare_dram_parameter("out", [D, Y, xs, Z], F16, isOutput=True)

    S_H = ZP           # pair-half stride (elements)
    S_X = 2 * ZP       # x-plane stride (elements)

    if xs == XS:
        chunks = CHUNKS
    else:
        chunks = [min(8, xs)] * max(1, xs // min(8, xs))
    cuts = [0]
    off = 0
    for kb in chunks:
        off += kb
        cuts.append(min(xh, off + 2))
    while len(cuts) >= 2 and cuts[-2] == cuts[-1]:
        cuts.pop()
    if cuts[-1] != xh:
        cuts.append(xh)

    def eng(kind, it, i):
        return nc.vector if ENG[kind](it, i) == "v" else nc.gpsimd

    def pair_shift_x(t, x0, kb):
        base = t[:, :, :, :]
        p = base.ap[0][0]
        off = base.offset + x0 * S_X + 1
        in0 = AP(base.tensor, off + 2 * S_X,
                 [[p, Y], [S_H - 2 * S_X, 2], [S_X, kb], [1, Z]])
        in1 = AP(base.tensor, off,
                 [[p, Y], [S_H + 2 * S_X, 2], [S_X, kb], [1, Z]])
        return in0, in1

    def pair_shift_z(t, x0, kb):
        base = t[:, :, :, :]
        p = base.ap[0][0]
        off = base.offset + (x0 + 1) * S_X
        in0 = AP(base.tensor, off + 2,
                 [[p, Y], [S_H - 2, 2], [S_X, kb], [1, Z]])
        in1 = AP(base.tensor, off,
                 [[p, Y], [S_H + 2, 2], [S_X, kb], [1, Z]])
        return in0, in1

    def pair_swap(t, x0, kb):
        base = t[:, :, :, :]
        p = base.ap[0][0]
        off = base.offset + S_H + (x0 + 1) * S_X + 1
        return AP(base.tensor, off,
                  [[p, Y], [-S_H, 2], [S_X, kb], [1, Z]])

    with tile.TileContext(nc) as tc:
        with (
            tc.tile_pool(name="inp", bufs=1) as inp,
            tc.tile_pool(name="wp", bufs=1) as wp,
            tc.tile_pool(name="dpool", bufs=3) as dpool,
            tc.tile_pool(name="ppool", bufs=3) as ppool,
            tc.tile_pool(name="ypool", bufs=3) as ypool,
            tc.tile_pool(name="spool", bufs=3) as spool,
            tc.tile_pool(name="opool", bufs=8) as opool,
            tc.tile_pool(name="psum_dy", bufs=2, space="PSUM") as psum_dy,
            tc.tile_pool(name="psum_acc", bufs=4, space="PSUM") as psum_acc,
        ):
            wt = wp.tile([Y, 4 * Y], F16, name="wt")
            dyT = wt[:, 0:Y]            # 0.25*DyT
            ndyT = wt[:, Y:2 * Y]       # -0.25*DyT
            eyeT = wt[:, 2 * Y:3 * Y]   # I
            qeyeT = wt[:, 3 * Y:4 * Y]  # 0.25*I

            ct = []
            for i in range(D):
                t = inp.tile([Y, xh, 2, ZP], F16, name=f"ch{i}", tag=f"ch{i}")
                ct.append(t)
            nc.sync.dma_start(out=wt[:, :], in_=w_h[:, :])
            for a, b2 in zip(cuts, cuts[1:]):
                for i in range(D):
                    nc.sync.dma_start(out=ct[i][:, a:b2, :, :],
                                      in_=in_h[i][:, a:b2, :, :])

            zc = slice(1, 1 + Z)

            # Prime PE's vector clock against every input DMA chunk.
            scratch = psum_acc.tile([8, 8], F32, name="scratch", tag="acc")
            for a in cuts[:-1]:
                for t in ct:
                    nc.tensor.matmul(scratch[:, 0:1], wt[:, 0:8],
                                     t[:, a:a + 1, 0:1, 0:1],
                                     start=True, stop=True)

            items = []
            off = 0
            for kb in chunks:
                items.append((off, kb))
                off += kb
            assert off == xs

            def stage_a(it, i):
                x0, kb = items[it]
                dX = dpool.tile([Y, 2, kb, Z], F16, name="dX", tag="dX")
                i0, i1 = pair_shift_x(ct[i], x0, kb)
                eng("dX", it, i).tensor_sub(out=dX[:, :, :, :], in0=i0, in1=i1)
                pX = ppool.tile([Y, 2, kb, Z], F16, name="pX", tag="pX")
                eng("pX", it, i).tensor_mul(out=pX[:, :, :, :],
                                            in0=dX[:, :, :, :],
                                            in1=pair_swap(ct[0], x0, kb))
                dZ = dpool.tile([Y, 2, kb, Z], F16, name="dZ", tag="dZ")
                j0, j1 = pair_shift_z(ct[i], x0, kb)
                eng("dZ", it, i).tensor_sub(out=dZ[:, :, :, :], in0=j0, in1=j1)
                pZ = ppool.tile([Y, 2, kb, Z], F16, name="pZ", tag="pZ")
                eng("pZ", it, i).tensor_mul(out=pZ[:, :, :, :],
                                            in0=dZ[:, :, :, :],
                                            in1=pair_swap(ct[2], x0, kb))
                if ENG["route"](it, i) == "c":
                    s = spool.tile([Y, kb, Z], F16, name="s", tag="s")
                    eng("s", it, i).tensor_add(
                        out=s[:, :, :],
                        in0=ct[i][:, 1 + x0:1 + x0 + kb, 0, zc],
                        in1=ct[i][:, 1 + x0:1 + x0 + kb, 1, zc])
                else:
                    s = None
                return pX, pZ, s

            def stage_y(it, i):
                x0, kb = items[it]
                dyS = ypool.tile([Y, 2, kb, Z], F16, name="dyS", tag="dyS")
                for ho, kx in acc_chunks(kb):
                    x0h = x0 + ho
                    hs = slice(1 + x0h, 1 + x0h + kx)
                    dyP = psum_dy.tile([Y, 2, kx, Z], F32,
                                       name="dyP", tag="dyP")
                    nc.tensor.matmul(dyP[:, 0, :, :], dyT,
                                     ct[i][:, hs, 0, zc],
                                     start=True, stop=True)
                    nc.tensor.matmul(dyP[:, 1, :, :], ndyT,
                                     ct[i][:, hs, 1, zc],
                                     start=True, stop=True)
                    nc.scalar.copy(out=dyS[:, :, ho:ho + kx, :],
                                   in_=dyP[:, :, :, :])
                pY = ppool.tile([Y, 2, kb, Z], F16, name="pY", tag="pY")
                eng("pY", it, i).tensor_mul(out=pY[:, :, :, :],
                                            in0=dyS[:, :, :, :],
                                            in1=pair_swap(ct[1], x0, kb))
                return pY

            def acc_chunks(kb):
                out = []
                o = 0
                while o < kb:
                    k = min(KX, kb - o)
                    out.append((o, k))
                    o += k
                return out

            def stage_b(it, i, h, pX, pZ, s, pY, ot):
                x0, kb = items[it]
                ho, kx = acc_chunks(kb)[h]
                ph = slice(ho, ho + kx)
                sig = ENG["route"](it, i) == "sigma"
                hs = slice(1 + x0 + ho, 1 + x0 + ho + kx)
                acc = psum_acc.tile([Y, kx, Z], F32, name="acc", tag="acc")
                nc.tensor.matmul(acc[:, :, :], qeyeT, pX[:, 0, ph, :],
                                 start=True, stop=False)
                nc.tensor.matmul(acc[:, :, :], qeyeT, pX[:, 1, ph, :],
                                 start=False, stop=False)
                nc.tensor.matmul(acc[:, :, :], qeyeT, pZ[:, 0, ph, :],
                                 start=False, stop=False)
                nc.tensor.matmul(acc[:, :, :], qeyeT, pZ[:, 1, ph, :],
                                 start=False, stop=False)
                nc.tensor.matmul(acc[:, :, :], eyeT, pY[:, 0, ph, :],
                                 start=False, stop=False)
                if sig:
                    nc.tensor.matmul(acc[:, :, :], eyeT, pY[:, 1, ph, :],
                                     start=False, stop=False)
                    nc.tensor.matmul(acc[:, :, :], eyeT,
                                     ct[i][:, hs, 0, zc],
                                     start=False, stop=False)
                    nc.tensor.matmul(acc[:, :, :], eyeT,
                                     ct[i][:, hs, 1, zc],
                                     start=False, stop=True)
                    nc.scalar.copy(out=ot[:, ph, :], in_=acc[:, :, :])
                else:
                    nc.tensor.matmul(acc[:, :, :], eyeT, pY[:, 1, ph, :],
                                     start=False, stop=True)
                    eng("evac", it, i).tensor_add(out=ot[:, ph, :],
                                                  in0=acc[:, :, :],
                                                  in1=s[:, ph, :])

            def nkx(it):
                x0, kb = items[it]
                return len(acc_chunks(kb))

            work = [(it, i) for it in range(len(items)) for i in range(D)]

            def run_b(entry):
                (pit, pi), (ppX, ppZ, ps), pys, pot = entry
                for h in range(nkx(pit)):
                    stage_b(pit, pi, h, ppX, ppZ, ps, pys, pot)
                px0, pkb = items[pit]
                oeng = nc.sync if ENG["odma"](pit, pi) == "s" else nc.scalar
                oeng.dma_start(out=out_h[pi, :, px0:px0 + pkb, :],
                               in_=pot[:, :, :])

            pending = []
            for it, i in work:
                cur_a = stage_a(it, i)
                cur_y = stage_y(it, i)
                x0, kb = items[it]
                ot = opool.tile([Y, kb, Z], F16, name="ot", tag="ot")
                pending.append(((it, i), cur_a, cur_y, ot))
                if len(pending) > DEPTH:
                    run_b(pending.pop(0))
            for entry in pending:
                run_b(entry)

    if not nc.is_finalized():
        nc.finalize()
    return nc


def _host_shard(l_b: np.ndarray, r_b: np.ndarray, xs: int) -> list[dict]:
    """(D, X, Y, Z) f32 pair -> per-slab dicts of (Y, 2, xs+2, ZP) fp16."""
    out = []
    for s0 in range(X // xs):
        idx = (np.arange(-1, xs + 1) + s0 * xs) % X
        m = {}
        for i in range(D):
            sl = np.stack([l_b[i][idx], r_b[i][idx]], axis=0)  # (2,xh,Y,Z)
            sl = np.transpose(sl, (2, 1, 0, 3))                # (Y,xh,2,Z)
            sl = np.concatenate([sl[..., Z - 1:Z], sl, sl[..., 0:1]], axis=-1)
            m[f"ch{i}"] = np.ascontiguousarray(sl.astype(np.float16))
        out.append(m)
    return out


def kernel(left: np.ndarray, right: np.ndarray) -> np.ndarray:
    left = np.asarray(left)
    right = np.asarray(right)
    assert left.shape == (B, D, X, Y, Z), left.shape

    wmats = _make_wmats()
    slabs = X // XS  # 4

    shards = [_host_shard(np.asarray(left[b], dtype=np.float32),
                          np.asarray(right[b], dtype=np.float32), XS)
              for b in range(B)]

    maps = []
    for core in range(NCORES):
        b, s0 = divmod(core, slabs)
        m = dict(shards[b][s0])
        m["wmats"] = wmats
        maps.append(m)

    nc = build_nc(XS)
    res = run_bass_kernel_spmd(nc, maps, core_ids=list(range(NCORES)))

    out = np.empty((B, D, X, Y, Z), dtype=np.float32)
    for core in range(NCORES):
        b, s0 = divmod(core, slabs)
        o = res.results[core]["out"].astype(np.float32)   # (D, Y, XS, Z)
        out[b, :, s0 * XS:(s0 + 1) * XS, :, :] = np.transpose(o, (0, 2, 1, 3))
    return out


# ---------------------------------------------------------------------------
# numpy reference of the same math (for probing without jax)
def _np_ref(left: np.ndarray, right: np.ndarray) -> np.ndarray:
    l = np.moveaxis(left, 1, -1).astype(np.float64)
    r = np.moveaxis(right, 1, -1).astype(np.float64)

    def jac(v):
        cols = []
        for j in range(3):
            ax = 1 + j
            g = (np.roll(v, -1, axis=ax) - np.roll(v, 1, axis=ax)) * 0.5
            cols.append(g)
        return np.stack(cols, axis=-1)

    jx, jy = jac(l), jac(r)
    br = np.einsum("bxyzij,bxyzj->bxyzi", jx, r) - np.einsum(
        "bxyzij,bxyzj->bxyzi", jy, l)
    z = l + r + 0.5 * br
    return np.moveaxis(z, -1, 1).astype(np.float32)


# revision 3
# speedup vs baseline: 1.1019x; 1.1019x over previous
"""Trainium2 Bass kernel v2 for truncated BCH on 3D vector fields.

Math (matches the jax reference):
  out_i = l_i + r_i + 0.25 * sum_j ( D_j l_i * r_j  -  D_j r_i * l_j )
where D_j v = v[.+1] - v[.-1] along spatial axis j (circulant wrap).

Design (v2):
  - Inputs arrive as per-channel PAIR tiles (Y, 2, xh, ZP) fp16 holding
    [l_i | r_i]; every x/z diff and product is ONE instruction covering
    both sides via strided access patterns:
      * diff pair op: in0[c,x] = t[c, x+1-2c], in1[c,x] = t[c, x-1+2c]
        -> [D l | -D r]  (r-side sign folded by the reversed stencil)
      * product pair op: multiplicand AP [r_j | l_j] read from channel-j's
        pair tile with a negative stride on the pair axis.
  - Y diffs on PE with weights [0.25*DyT | -0.25*DyT] into one PSUM pair
    tile; ScalarE evacuates to fp16; DVE multiplies by [r_1 | l_1].
  - Products accumulate into a single-bank PSUM acc via 0.25I / I weight
    matmuls (0.25 pre-folded, so no final scale op is needed).
  - Linear term: s_i = l_i + r_i on DVE/Pool; the final evac is ONE fused
    tensor_add: out_fp16 = acc_psum + s (no PE inject, no scale pass).
  - Output is written fp16 (host upcasts); rel-err stays ~1e-3.
"""

import sys

sys.path.insert(0, "/opt/trn_rl_repo")

import numpy as np

import concourse.bass as bass
import concourse.bacc as bacc
import concourse.mybir as mybir
import concourse.tile as tile
from concourse.ap import AP
from concourse.bass_utils import run_bass_kernel_spmd

B, D, X, Y, Z = 2, 3, 128, 128, 128
NCORES = 8
XS = (B * X) // NCORES      # 32 output x-planes per core
XH = XS + 2                 # with 1-plane halo each side
ZP = Z + 2                  # z padded: [z127, z0..z127, z0]
KX = 4                      # x-planes per PSUM acc chunk (512 f32 = 1 bank)
CHUNKS = [2, 6, 8, 8, 4, 2, 2]  # x-planes per compute work item
DEPTH = 2                   # stage_b pipeline lag (work items)

F16 = mybir.dt.float16
F32 = mybir.dt.float32


def _make_wmats() -> np.ndarray:
    """[0.25*DyT | -0.25*DyT | I | 0.25*I] as one (Y, 4Y) fp16 matrix."""
    e = np.eye(Y, dtype=np.float32)
    dy = np.roll(e, -1, axis=0) - np.roll(e, 1, axis=0)
    dyt = dy.T
    mats = np.concatenate([0.25 * dyt, -0.25 * dyt, e, 0.25 * e], axis=1)
    return mats.astype(np.float16)


# engine assignment per op kind; value is a function(item_idx, ch) -> engine
# name "v" (DVE) or "g" (Pool).  "route" selects the linear-term path per
# work item: "c" = s-add + fused evac-add (DVE/Pool), "sigma" = inject l,r
# via I-weight matmuls on PE + plain ScalarE copy evac.  Tuned by sweeping.
ENG = {
    "dX": lambda it, i: "v",
    "pX": lambda it, i: "v",
    "dZ": lambda it, i: "g",
    "pZ": lambda it, i: "g",
    "pY": lambda it, i: "v",
    "s":  lambda it, i: "v",
    "evac": lambda it, i: "g",
    "route": lambda it, i: "sigma" if (it + i) % 2 == 0 else "c",
    "odma": lambda it, i: "s",
}


def build_nc(xs: int = XS) -> bass.Bass:
    xh = xs + 2
    nc = bacc.Bacc(None)

    in_h = [nc.declare_dram_parameter(f"ch{i}", [Y, xh, 2, ZP], F16,
                                      isOutput=False) for i in range(D)]
    w_h = nc.declare_dram_parameter("wmats", [Y, 4 * Y], F16, isOutput=False)
    out_h = nc.declare_dram_parameter("out", [D, Y, xs, Z], F16, isOutput=True)

    S_H = ZP           # pair-half stride (elements)
    S_X = 2 * ZP       # x-plane stride (elements)

    if xs == XS:
        chunks = CHUNKS
    else:
        chunks = [min(8, xs)] * max(1, xs // min(8, xs))
    cuts = [0]
    off = 0
    for kb in chunks:
        off += kb
        cuts.append(min(xh, off + 2))
    while len(cuts) >= 2 and cuts[-2] == cuts[-1]:
        cuts.pop()
    if cuts[-1] != xh:
        cuts.append(xh)

    def eng(kind, it, i):
        return nc.vector if ENG[kind](it, i) == "v" else nc.gpsimd

    def pair_shift_x(t, x0, kb):
        base = t[:, :, :, :]
        p = base.ap[0][0]
        off = base.offset + x0 * S_X + 1
        in0 = AP(base.tensor, off + 2 * S_X,
                 [[p, Y], [S_H - 2 * S_X, 2], [S_X, kb], [1, Z]])
        in1 = AP(base.tensor, off,
                 [[p, Y], [S_H + 2 * S_X, 2], [S_X, kb], [1, Z]])
        return in0, in1

    def pair_shift_z(t, x0, kb):
        base = t[:, :, :, :]
        p = base.ap[0][0]
        off = base.offset + (x0 + 1) * S_X
        in0 = AP(base.tensor, off + 2,
                 [[p, Y], [S_H - 2, 2], [S_X, kb], [1, Z]])
        in1 = AP(base.tensor, off,
                 [[p, Y], [S_H + 2, 2], [S_X, kb], [1, Z]])
        return in0, in1

    def pair_swap(t, x0, kb):
        base = t[:, :, :, :]
        p = base.ap[0][0]
        off = base.offset + S_H + (x0 + 1) * S_X + 1
        return AP(base.tensor, off,
                  [[p, Y], [-S_H, 2], [S_X, kb], [1, Z]])

    with tile.TileContext(nc) as tc:
        with (
            tc.tile_pool(name="inp", bufs=1) as inp,
            tc.tile_pool(name="wp", bufs=1) as wp,
            tc.tile_pool(name="dpool", bufs=3) as dpool,
            tc.tile_pool(name="ppool", bufs=3) as ppool,
            tc.tile_pool(name="ypool", bufs=3) as ypool,
            tc.tile_pool(name="spool", bufs=3) as spool,
            tc.tile_pool(name="opool", bufs=8) as opool,
            tc.tile_pool(name="psum_dy", bufs=2, space="PSUM") as psum_dy,
            tc.tile_pool(name="psum_acc", bufs=4, space="PSUM") as psum_acc,
        ):
            wt = wp.tile([Y, 4 * Y], F16, name="wt")
            dyT = wt[:, 0:Y]            # 0.25*DyT
            ndyT = wt[:, Y:2 * Y]       # -0.25*DyT
            eyeT = wt[:, 2 * Y:3 * Y]   # I
            qeyeT = wt[:, 3 * Y:4 * Y]  # 0.25*I

            ct = []
            for i in range(D):
                t = inp.tile([Y, xh, 2, ZP], F16, name=f"ch{i}", tag=f"ch{i}")
                ct.append(t)
            nc.sync.dma_start(out=wt[:, :], in_=w_h[:, :])
            for a, b2 in zip(cuts, cuts[1:]):
                for i in range(D):
                    nc.sync.dma_start(out=ct[i][:, a:b2, :, :],
                                      in_=in_h[i][:, a:b2, :, :])

            zc = slice(1, 1 + Z)

            # Prime PE's vector clock against every input DMA chunk.
            scratch = psum_acc.tile([8, 8], F32, name="scratch", tag="acc")
            for a in cuts[:-1]:
                for t in ct:
                    nc.tensor.matmul(scratch[:, 0:1], wt[:, 0:8],
                                     t[:, a:a + 1, 0:1, 0:1],
                                     start=True, stop=True)

            items = []
            off = 0
            for kb in chunks:
                items.append((off, kb))
                off += kb
            assert off == xs

            def stage_a(it, i):
                x0, kb = items[it]
                dX = dpool.tile([Y, 2, kb, Z], F16, name="dX", tag="dX")
                i0, i1 = pair_shift_x(ct[i], x0, kb)
                eng("dX", it, i).tensor_sub(out=dX[:, :, :, :], in0=i0, in1=i1)
                pX = ppool.tile([Y, 2, kb, Z], F16, name="pX", tag="pX")
                eng("pX", it, i).tensor_mul(out=pX[:, :, :, :],
                                            in0=dX[:, :, :, :],
                                            in1=pair_swap(ct[0], x0, kb))
                dZ = dpool.tile([Y, 2, kb, Z], F16, name="dZ", tag="dZ")
                j0, j1 = pair_shift_z(ct[i], x0, kb)
                eng("dZ", it, i).tensor_sub(out=dZ[:, :, :, :], in0=j0, in1=j1)
                pZ = ppool.tile([Y, 2, kb, Z], F16, name="pZ", tag="pZ")
                eng("pZ", it, i).tensor_mul(out=pZ[:, :, :, :],
                                            in0=dZ[:, :, :, :],
                                            in1=pair_swap(ct[2], x0, kb))
                if ENG["route"](it, i) not in ("sigma", "r8"):
                    s = spool.tile([Y, kb, Z], F16, name="s", tag="s")
                    eng("s", it, i).tensor_add(
                        out=s[:, :, :],
                        in0=ct[i][:, 1 + x0:1 + x0 + kb, 0, zc],
                        in1=ct[i][:, 1 + x0:1 + x0 + kb, 1, zc])
                else:
                    s = None
                return pX, pZ, s

            def stage_y(it, i):
                x0, kb = items[it]
                dyS = ypool.tile([Y, 2, kb, Z], F16, name="dyS", tag="dyS")
                for ho, kx in acc_chunks(kb):
                    x0h = x0 + ho
                    hs = slice(1 + x0h, 1 + x0h + kx)
                    dyP = psum_dy.tile([Y, 2, kx, Z], F32,
                                       name="dyP", tag="dyP")
                    nc.tensor.matmul(dyP[:, 0, :, :], dyT,
                                     ct[i][:, hs, 0, zc],
                                     start=True, stop=True)
                    nc.tensor.matmul(dyP[:, 1, :, :], ndyT,
                                     ct[i][:, hs, 1, zc],
                                     start=True, stop=True)
                    nc.scalar.copy(out=dyS[:, :, ho:ho + kx, :],
                                   in_=dyP[:, :, :, :])
                pY = ppool.tile([Y, 2, kb, Z], F16, name="pY", tag="pY")
                eng("pY", it, i).tensor_mul(out=pY[:, :, :, :],
                                            in0=dyS[:, :, :, :],
                                            in1=pair_swap(ct[1], x0, kb))
                return pY

            def acc_chunks(kb):
                out = []
                o = 0
                while o < kb:
                    k = min(KX, kb - o)
                    out.append((o, k))
                    o += k
                return out

            def stage_b(it, i, h, pX, pZ, s, pY, ot):
                x0, kb = items[it]
                ho, kx = acc_chunks(kb)[h]
                ph = slice(ho, ho + kx)
                sig = ENG["route"](it, i) in ("sigma", "r8")
                hs = slice(1 + x0 + ho, 1 + x0 + ho + kx)
                acc = psum_acc.tile([Y, kx, Z], F32, name="acc", tag="acc")
                nc.tensor.matmul(acc[:, :, :], qeyeT, pX[:, 0, ph, :],
                                 start=True, stop=False)
                nc.tensor.matmul(acc[:, :, :], qeyeT, pX[:, 1, ph, :],
                                 start=False, stop=False)
                nc.tensor.matmul(acc[:, :, :], qeyeT, pZ[:, 0, ph, :],
                                 start=False, stop=False)
                nc.tensor.matmul(acc[:, :, :], qeyeT, pZ[:, 1, ph, :],
                                 start=False, stop=False)
                nc.tensor.matmul(acc[:, :, :], eyeT, pY[:, 0, ph, :],
                                 start=False, stop=False)
                if sig:
                    nc.tensor.matmul(acc[:, :, :], eyeT, pY[:, 1, ph, :],
                                     start=False, stop=False)
                    nc.tensor.matmul(acc[:, :, :], eyeT,
                                     ct[i][:, hs, 0, zc],
                                     start=False, stop=False)
                    nc.tensor.matmul(acc[:, :, :], eyeT,
                                     ct[i][:, hs, 1, zc],
                                     start=False, stop=True)
                    nc.scalar.copy(out=ot[:, ph, :], in_=acc[:, :, :])
                else:
                    nc.tensor.matmul(acc[:, :, :], eyeT, pY[:, 1, ph, :],
                                     start=False, stop=True)
                    # HW: GPSIMD cannot touch PSUM -> Act evacuates, then
                    # a cheap fp16 2x add folds in the linear term.
                    nc.scalar.copy(out=ot[:, ph, :], in_=acc[:, :, :])
                    eng("evac", it, i).tensor_add(out=ot[:, ph, :],
                                                  in0=ot[:, ph, :],
                                                  in1=s[:, ph, :])

            def nkx(it):
                x0, kb = items[it]
                return len(acc_chunks(kb))

            work = [(it, i) for it in range(len(items)) for i in range(D)]

            def run_b(entry):
                (pit, pi), (ppX, ppZ, ps), pys, pot = entry
                for h in range(nkx(pit)):
                    stage_b(pit, pi, h, ppX, ppZ, ps, pys, pot)
                px0, pkb = items[pit]
                oeng = nc.sync if ENG["odma"](pit, pi) == "s" else nc.scalar
                oeng.dma_start(out=out_h[pi, :, px0:px0 + pkb, :],
                               in_=pot[:, :, :])

            pending = []
            for it, i in work:
                cur_a = stage_a(it, i)
                cur_y = stage_y(it, i)
                x0, kb = items[it]
                ot = opool.tile([Y, kb, Z], F16, name="ot", tag="ot")
                pending.append(((it, i), cur_a, cur_y, ot))
                if len(pending) > DEPTH:
                    run_b(pending.pop(0))
            for entry in pending:
                run_b(entry)

    if not nc.is_finalized():
        nc.finalize()
    return nc


def _host_shard(l_b: np.ndarray, r_b: np.ndarray, xs: int) -> list[dict]:
    """(D, X, Y, Z) f32 pair -> per-slab dicts of (Y, 2, xs+2, ZP) fp16."""
    out = []
    for s0 in range(X // xs):
        idx = (np.arange(-1, xs + 1) + s0 * xs) % X
        m = {}
        for i in range(D):
            sl = np.stack([l_b[i][idx], r_b[i][idx]], axis=0)  # (2,xh,Y,Z)
            sl = np.transpose(sl, (2, 1, 0, 3))                # (Y,xh,2,Z)
            sl = np.concatenate([sl[..., Z - 1:Z], sl, sl[..., 0:1]], axis=-1)
            m[f"ch{i}"] = np.ascontiguousarray(sl.astype(np.float16))
        out.append(m)
    return out


def kernel(left: np.ndarray, right: np.ndarray) -> np.ndarray:
    left = np.asarray(left)
    right = np.asarray(right)
    assert left.shape == (B, D, X, Y, Z), left.shape

    wmats = _make_wmats()
    slabs = X // XS  # 4

    shards = [_host_shard(np.asarray(left[b], dtype=np.float32),
                          np.asarray(right[b], dtype=np.float32), XS)
              for b in range(B)]

    maps = []
    for core in range(NCORES):
        b, s0 = divmod(core, slabs)
        m = dict(shards[b][s0])
        m["wmats"] = wmats
        maps.append(m)

    nc = build_nc(XS)
    res = run_bass_kernel_spmd(nc, maps, core_ids=list(range(NCORES)))

    out = np.empty((B, D, X, Y, Z), dtype=np.float32)
    for core in range(NCORES):
        b, s0 = divmod(core, slabs)
        o = res.results[core]["out"].astype(np.float32)   # (D, Y, XS, Z)
        out[b, :, s0 * XS:(s0 + 1) * XS, :, :] = np.transpose(o, (0, 2, 1, 3))
    return out


# ---------------------------------------------------------------------------
# numpy reference of the same math (for probing without jax)
def _np_ref(left: np.ndarray, right: np.ndarray) -> np.ndarray:
    l = np.moveaxis(left, 1, -1).astype(np.float64)
    r = np.moveaxis(right, 1, -1).astype(np.float64)

    def jac(v):
        cols = []
        for j in range(3):
            ax = 1 + j
            g = (np.roll(v, -1, axis=ax) - np.roll(v, 1, axis=ax)) * 0.5
            cols.append(g)
        return np.stack(cols, axis=-1)

    jx, jy = jac(l), jac(r)
    br = np.einsum("bxyzij,bxyzj->bxyzi", jx, r) - np.einsum(
        "bxyzij,bxyzj->bxyzi", jy, l)
    z = l + r + 0.5 * br
    return np.moveaxis(z, -1, 1).astype(np.float32)


# revision 4
# speedup vs baseline: 1.1156x; 1.0124x over previous
"""Trainium2 Bass kernel v2 for truncated BCH on 3D vector fields.

Math (matches the jax reference):
  out_i = l_i + r_i + 0.25 * sum_j ( D_j l_i * r_j  -  D_j r_i * l_j )
where D_j v = v[.+1] - v[.-1] along spatial axis j (circulant wrap).

Design (v2):
  - Inputs arrive as per-channel PAIR tiles (Y, 2, xh, ZP) fp16 holding
    [l_i | r_i]; every x/z diff and product is ONE instruction covering
    both sides via strided access patterns:
      * diff pair op: in0[c,x] = t[c, x+1-2c], in1[c,x] = t[c, x-1+2c]
        -> [D l | -D r]  (r-side sign folded by the reversed stencil)
      * product pair op: multiplicand AP [r_j | l_j] read from channel-j's
        pair tile with a negative stride on the pair axis.
  - Y diffs on PE with weights [0.25*DyT | -0.25*DyT] into one PSUM pair
    tile; ScalarE evacuates to fp16; DVE multiplies by [r_1 | l_1].
  - Products accumulate into a single-bank PSUM acc via 0.25I / I weight
    matmuls (0.25 pre-folded, so no final scale op is needed).
  - Linear term: s_i = l_i + r_i on DVE/Pool; the final evac is ONE fused
    tensor_add: out_fp16 = acc_psum + s (no PE inject, no scale pass).
  - Output is written fp16 (host upcasts); rel-err stays ~1e-3.
"""

import sys

sys.path.insert(0, "/opt/trn_rl_repo")

import numpy as np

import concourse.bass as bass
import concourse.bacc as bacc
import concourse.mybir as mybir
import concourse.tile as tile
from concourse.ap import AP
from concourse.bass_utils import run_bass_kernel_spmd

B, D, X, Y, Z = 2, 3, 128, 128, 128
NCORES = 8
XS = (B * X) // NCORES      # 32 output x-planes per core
XH = XS + 2                 # with 1-plane halo each side
ZP = Z + 2                  # z padded: [z127, z0..z127, z0]
KX = 4                      # x-planes per PSUM acc chunk (512 f32 = 1 bank)
CHUNKS = [2, 6, 8, 8, 4, 2, 2]  # x-planes per compute work item
DEPTH = 2                   # stage_b pipeline lag (work items)

F16 = mybir.dt.float16
F32 = mybir.dt.float32


def _make_wmats() -> np.ndarray:
    """[0.25*DyT | -0.25*DyT | I | 0.25*I] as one (Y, 4Y) fp16 matrix."""
    e = np.eye(Y, dtype=np.float32)
    dy = np.roll(e, -1, axis=0) - np.roll(e, 1, axis=0)
    dyt = dy.T
    mats = np.concatenate([0.25 * dyt, -0.25 * dyt, e, 0.25 * e], axis=1)
    return mats.astype(np.float16)


# engine assignment per op kind; value is a function(item_idx, ch) -> engine
# name "v" (DVE) or "g" (Pool).  "route" selects the linear-term path per
# work item: "c" = s-add + fused evac-add (DVE/Pool), "sigma" = inject l,r
# via I-weight matmuls on PE + plain ScalarE copy evac.  Tuned by sweeping.
ENG = {
    "dX": lambda it, i: "v",
    "pX": lambda it, i: "v",
    "dZ": lambda it, i: "g",
    "pZ": lambda it, i: "g",
    "pY": lambda it, i: "v",
    "s":  lambda it, i: "v",
    "evac": lambda it, i: "g",
    "route": lambda it, i: "sigma" if (i == 2 or it in (0, 6)) else "c",
    "odma": lambda it, i: "s",
}


def build_nc(xs: int = XS) -> bass.Bass:
    xh = xs + 2
    nc = bacc.Bacc(None)

    in_h = [nc.declare_dram_parameter(f"ch{i}", [Y, xh, 2, ZP], F16,
                                      isOutput=False) for i in range(D)]
    w_h = nc.declare_dram_parameter("wmats", [Y, 4 * Y], F16, isOutput=False)
    out_h = nc.declare_dram_parameter("out", [D, Y, xs, Z], F16, isOutput=True)

    S_H = ZP           # pair-half stride (elements)
    S_X = 2 * ZP       # x-plane stride (elements)

    if xs == XS:
        chunks = CHUNKS
    else:
        chunks = [min(8, xs)] * max(1, xs // min(8, xs))
    cuts = [0]
    off = 0
    for kb in chunks:
        off += kb
        cuts.append(min(xh, off + 2))
    while len(cuts) >= 2 and cuts[-2] == cuts[-1]:
        cuts.pop()
    if cuts[-1] != xh:
        cuts.append(xh)

    def eng(kind, it, i):
        return nc.vector if ENG[kind](it, i) == "v" else nc.gpsimd

    def pair_shift_x(t, x0, kb):
        base = t[:, :, :, :]
        p = base.ap[0][0]
        off = base.offset + x0 * S_X + 1
        in0 = AP(base.tensor, off + 2 * S_X,
                 [[p, Y], [S_H - 2 * S_X, 2], [S_X, kb], [1, Z]])
        in1 = AP(base.tensor, off,
                 [[p, Y], [S_H + 2 * S_X, 2], [S_X, kb], [1, Z]])
        return in0, in1

    def pair_shift_z(t, x0, kb):
        base = t[:, :, :, :]
        p = base.ap[0][0]
        off = base.offset + (x0 + 1) * S_X
        in0 = AP(base.tensor, off + 2,
                 [[p, Y], [S_H - 2, 2], [S_X, kb], [1, Z]])
        in1 = AP(base.tensor, off,
                 [[p, Y], [S_H + 2, 2], [S_X, kb], [1, Z]])
        return in0, in1

    def pair_swap(t, x0, kb):
        base = t[:, :, :, :]
        p = base.ap[0][0]
        off = base.offset + S_H + (x0 + 1) * S_X + 1
        return AP(base.tensor, off,
                  [[p, Y], [-S_H, 2], [S_X, kb], [1, Z]])

    with tile.TileContext(nc) as tc:
        with (
            tc.tile_pool(name="inp", bufs=1) as inp,
            tc.tile_pool(name="wp", bufs=1) as wp,
            tc.tile_pool(name="dpool", bufs=3) as dpool,
            tc.tile_pool(name="ppool", bufs=3) as ppool,
            tc.tile_pool(name="ypool", bufs=3) as ypool,
            tc.tile_pool(name="spool", bufs=3) as spool,
            tc.tile_pool(name="opool", bufs=8) as opool,
            tc.tile_pool(name="psum_dy", bufs=2, space="PSUM") as psum_dy,
            tc.tile_pool(name="psum_acc", bufs=4, space="PSUM") as psum_acc,
        ):
            wt = wp.tile([Y, 4 * Y], F16, name="wt")
            dyT = wt[:, 0:Y]            # 0.25*DyT
            ndyT = wt[:, Y:2 * Y]       # -0.25*DyT
            eyeT = wt[:, 2 * Y:3 * Y]   # I
            qeyeT = wt[:, 3 * Y:4 * Y]  # 0.25*I

            ct = []
            for i in range(D):
                t = inp.tile([Y, xh, 2, ZP], F16, name=f"ch{i}", tag=f"ch{i}")
                ct.append(t)
            nc.sync.dma_start(out=wt[:, :], in_=w_h[:, :])
            for a, b2 in zip(cuts, cuts[1:]):
                for i in range(D):
                    nc.sync.dma_start(out=ct[i][:, a:b2, :, :],
                                      in_=in_h[i][:, a:b2, :, :])

            zc = slice(1, 1 + Z)

            # Prime PE's vector clock against every input DMA chunk.
            scratch = psum_acc.tile([8, 8], F32, name="scratch", tag="acc")
            for a in cuts[:-1]:
                for t in ct:
                    nc.tensor.matmul(scratch[:, 0:1], wt[:, 0:8],
                                     t[:, a:a + 1, 0:1, 0:1],
                                     start=True, stop=True)

            items = []
            off = 0
            for kb in chunks:
                items.append((off, kb))
                off += kb
            assert off == xs

            def stage_a(it, i):
                x0, kb = items[it]
                dX = dpool.tile([Y, 2, kb, Z], F16, name="dX", tag="dX")
                i0, i1 = pair_shift_x(ct[i], x0, kb)
                eng("dX", it, i).tensor_sub(out=dX[:, :, :, :], in0=i0, in1=i1)
                pX = ppool.tile([Y, 2, kb, Z], F16, name="pX", tag="pX")
                eng("pX", it, i).tensor_mul(out=pX[:, :, :, :],
                                            in0=dX[:, :, :, :],
                                            in1=pair_swap(ct[0], x0, kb))
                dZ = dpool.tile([Y, 2, kb, Z], F16, name="dZ", tag="dZ")
                j0, j1 = pair_shift_z(ct[i], x0, kb)
                eng("dZ", it, i).tensor_sub(out=dZ[:, :, :, :], in0=j0, in1=j1)
                pZ = ppool.tile([Y, 2, kb, Z], F16, name="pZ", tag="pZ")
                eng("pZ", it, i).tensor_mul(out=pZ[:, :, :, :],
                                            in0=dZ[:, :, :, :],
                                            in1=pair_swap(ct[2], x0, kb))
                if ENG["route"](it, i) not in ("sigma", "r8"):
                    s = spool.tile([Y, kb, Z], F16, name="s", tag="s")
                    eng("s", it, i).tensor_add(
                        out=s[:, :, :],
                        in0=ct[i][:, 1 + x0:1 + x0 + kb, 0, zc],
                        in1=ct[i][:, 1 + x0:1 + x0 + kb, 1, zc])
                else:
                    s = None
                return pX, pZ, s

            def stage_y(it, i):
                x0, kb = items[it]
                dyS = ypool.tile([Y, 2, kb, Z], F16, name="dyS", tag="dyS")
                for ho, kx in acc_chunks(kb):
                    x0h = x0 + ho
                    hs = slice(1 + x0h, 1 + x0h + kx)
                    dyP = psum_dy.tile([Y, 2, kx, Z], F32,
                                       name="dyP", tag="dyP")
                    nc.tensor.matmul(dyP[:, 0, :, :], dyT,
                                     ct[i][:, hs, 0, zc],
                                     start=True, stop=True)
                    nc.tensor.matmul(dyP[:, 1, :, :], ndyT,
                                     ct[i][:, hs, 1, zc],
                                     start=True, stop=True)
                    nc.scalar.copy(out=dyS[:, :, ho:ho + kx, :],
                                   in_=dyP[:, :, :, :])
                pY = ppool.tile([Y, 2, kb, Z], F16, name="pY", tag="pY")
                eng("pY", it, i).tensor_mul(out=pY[:, :, :, :],
                                            in0=dyS[:, :, :, :],
                                            in1=pair_swap(ct[1], x0, kb))
                return pY

            def acc_chunks(kb):
                out = []
                o = 0
                while o < kb:
                    k = min(KX, kb - o)
                    out.append((o, k))
                    o += k
                return out

            def stage_b(it, i, h, pX, pZ, s, pY, ot):
                x0, kb = items[it]
                ho, kx = acc_chunks(kb)[h]
                ph = slice(ho, ho + kx)
                sig = ENG["route"](it, i) in ("sigma", "r8")
                hs = slice(1 + x0 + ho, 1 + x0 + ho + kx)
                acc = psum_acc.tile([Y, kx, Z], F32, name="acc", tag="acc")
                nc.tensor.matmul(acc[:, :, :], qeyeT, pX[:, 0, ph, :],
                                 start=True, stop=False)
                nc.tensor.matmul(acc[:, :, :], qeyeT, pX[:, 1, ph, :],
                                 start=False, stop=False)
                nc.tensor.matmul(acc[:, :, :], qeyeT, pZ[:, 0, ph, :],
                                 start=False, stop=False)
                nc.tensor.matmul(acc[:, :, :], qeyeT, pZ[:, 1, ph, :],
                                 start=False, stop=False)
                nc.tensor.matmul(acc[:, :, :], eyeT, pY[:, 0, ph, :],
                                 start=False, stop=False)
                if sig:
                    nc.tensor.matmul(acc[:, :, :], eyeT, pY[:, 1, ph, :],
                                     start=False, stop=False)
                    nc.tensor.matmul(acc[:, :, :], eyeT,
                                     ct[i][:, hs, 0, zc],
                                     start=False, stop=False)
                    nc.tensor.matmul(acc[:, :, :], eyeT,
                                     ct[i][:, hs, 1, zc],
                                     start=False, stop=True)
                    nc.scalar.copy(out=ot[:, ph, :], in_=acc[:, :, :])
                else:
                    nc.tensor.matmul(acc[:, :, :], eyeT, pY[:, 1, ph, :],
                                     start=False, stop=True)
                    # HW: GPSIMD cannot touch PSUM -> Act evacuates, then
                    # a cheap fp16 2x add folds in the linear term.
                    nc.scalar.copy(out=ot[:, ph, :], in_=acc[:, :, :])
                    eng("evac", it, i).tensor_add(out=ot[:, ph, :],
                                                  in0=ot[:, ph, :],
                                                  in1=s[:, ph, :])

            def nkx(it):
                x0, kb = items[it]
                return len(acc_chunks(kb))

            work = [(it, i) for it in range(len(items)) for i in range(D)]

            def run_b(entry):
                (pit, pi), (ppX, ppZ, ps), pys, pot = entry
                for h in range(nkx(pit)):
                    stage_b(pit, pi, h, ppX, ppZ, ps, pys, pot)
                px0, pkb = items[pit]
                oeng = nc.sync if ENG["odma"](pit, pi) == "s" else nc.scalar
                oeng.dma_start(out=out_h[pi, :, px0:px0 + pkb, :],
                               in_=pot[:, :, :])

            pending = []
            for it, i in work:
                cur_a = stage_a(it, i)
                cur_y = stage_y(it, i)
                x0, kb = items[it]
                ot = opool.tile([Y, kb, Z], F16, name="ot", tag="ot")
                pending.append(((it, i), cur_a, cur_y, ot))
                if len(pending) > DEPTH:
                    run_b(pending.pop(0))
            for entry in pending:
                run_b(entry)

    if not nc.is_finalized():
        nc.finalize()
    return nc


def _host_shard(l_b: np.ndarray, r_b: np.ndarray, xs: int) -> list[dict]:
    """(D, X, Y, Z) f32 pair -> per-slab dicts of (Y, 2, xs+2, ZP) fp16."""
    out = []
    for s0 in range(X // xs):
        idx = (np.arange(-1, xs + 1) + s0 * xs) % X
        m = {}
        for i in range(D):
            sl = np.stack([l_b[i][idx], r_b[i][idx]], axis=0)  # (2,xh,Y,Z)
            sl = np.transpose(sl, (2, 1, 0, 3))                # (Y,xh,2,Z)
            sl = np.concatenate([sl[..., Z - 1:Z], sl, sl[..., 0:1]], axis=-1)
            m[f"ch{i}"] = np.ascontiguousarray(sl.astype(np.float16))
        out.append(m)
    return out


def kernel(left: np.ndarray, right: np.ndarray) -> np.ndarray:
    left = np.asarray(left)
    right = np.asarray(right)
    assert left.shape == (B, D, X, Y, Z), left.shape

    wmats = _make_wmats()
    slabs = X // XS  # 4

    shards = [_host_shard(np.asarray(left[b], dtype=np.float32),
                          np.asarray(right[b], dtype=np.float32), XS)
              for b in range(B)]

    maps = []
    for core in range(NCORES):
        b, s0 = divmod(core, slabs)
        m = dict(shards[b][s0])
        m["wmats"] = wmats
        maps.append(m)

    nc = build_nc(XS)
    res = run_bass_kernel_spmd(nc, maps, core_ids=list(range(NCORES)))

    out = np.empty((B, D, X, Y, Z), dtype=np.float32)
    for core in range(NCORES):
        b, s0 = divmod(core, slabs)
        o = res.results[core]["out"].astype(np.float32)   # (D, Y, XS, Z)
        out[b, :, s0 * XS:(s0 + 1) * XS, :, :] = np.transpose(o, (0, 2, 1, 3))
    return out


# ---------------------------------------------------------------------------
# numpy reference of the same math (for probing without jax)
def _np_ref(left: np.ndarray, right: np.ndarray) -> np.ndarray:
    l = np.moveaxis(left, 1, -1).astype(np.float64)
    r = np.moveaxis(right, 1, -1).astype(np.float64)

    def jac(v):
        cols = []
        for j in range(3):
            ax = 1 + j
            g = (np.roll(v, -1, axis=ax) - np.roll(v, 1, axis=ax)) * 0.5
            cols.append(g)
        return np.stack(cols, axis=-1)

    jx, jy = jac(l), jac(r)
    br = np.einsum("bxyzij,bxyzj->bxyzi", jx, r) - np.einsum(
        "bxyzij,bxyzj->bxyzi", jy, l)
    z = l + r + 0.5 * br
    return np.moveaxis(z, -1, 1).astype(np.float32)


# revision 5
# speedup vs baseline: 1.1264x; 1.0097x over previous
"""Trainium2 Bass kernel v2 for truncated BCH on 3D vector fields.

Math (matches the jax reference):
  out_i = l_i + r_i + 0.25 * sum_j ( D_j l_i * r_j  -  D_j r_i * l_j )
where D_j v = v[.+1] - v[.-1] along spatial axis j (circulant wrap).

Design (v2):
  - Inputs arrive as per-channel PAIR tiles (Y, 2, xh, ZP) fp16 holding
    [l_i | r_i]; every x/z diff and product is ONE instruction covering
    both sides via strided access patterns:
      * diff pair op: in0[c,x] = t[c, x+1-2c], in1[c,x] = t[c, x-1+2c]
        -> [D l | -D r]  (r-side sign folded by the reversed stencil)
      * product pair op: multiplicand AP [r_j | l_j] read from channel-j's
        pair tile with a negative stride on the pair axis.
  - Y diffs on PE with weights [0.25*DyT | -0.25*DyT] into one PSUM pair
    tile; ScalarE evacuates to fp16; DVE multiplies by [r_1 | l_1].
  - Products accumulate into a single-bank PSUM acc via 0.25I / I weight
    matmuls (0.25 pre-folded, so no final scale op is needed).
  - Linear term, two routes tuned per work item ("route" table):
      "c":     s_i = l_i + r_i on DVE; ScalarE copies acc->out tile fp16;
               Pool (or DVE) adds s in-place (fp16 2x) - GPSIMD never
               touches PSUM (the HW verifier forbids it).
      "sigma": l_i and r_i injected straight into the acc via I-weight
               matmuls on PE; ScalarE copy is the whole evac.
  - Output is written fp16 (host upcasts); rel-err stays ~4e-4.
  - Engine tables (ENG) + work-item chunking (CHUNKS) were tuned with a
    noexec-CoreSim sweep and a per-item hill-climb; all five queues sit
    at 79-87% busy.
"""

import sys

sys.path.insert(0, "/opt/trn_rl_repo")

import numpy as np

import concourse.bass as bass
import concourse.bacc as bacc
import concourse.mybir as mybir
import concourse.tile as tile
from concourse.ap import AP
from concourse.bass_utils import run_bass_kernel_spmd

B, D, X, Y, Z = 2, 3, 128, 128, 128
NCORES = 8
XS = (B * X) // NCORES      # 32 output x-planes per core
XH = XS + 2                 # with 1-plane halo each side
ZP = Z + 2                  # z padded: [z127, z0..z127, z0]
KX = 4                      # x-planes per PSUM acc chunk (512 f32 = 1 bank)
CHUNKS = [2, 6, 8, 8, 4, 2, 2]  # x-planes per compute work item
DEPTH = 2                   # stage_b pipeline lag (work items)

F16 = mybir.dt.float16
F32 = mybir.dt.float32


def _make_wmats() -> np.ndarray:
    """[0.25*DyT | -0.25*DyT | I | 0.25*I] as one (Y, 4Y) fp16 matrix."""
    e = np.eye(Y, dtype=np.float32)
    dy = np.roll(e, -1, axis=0) - np.roll(e, 1, axis=0)
    dyt = dy.T
    mats = np.concatenate([0.25 * dyt, -0.25 * dyt, e, 0.25 * e], axis=1)
    return mats.astype(np.float16)


# engine assignment per op kind; value is a function(item_idx, ch) -> engine
# name "v" (DVE) or "g" (Pool).  "route" selects the linear-term path per
# work item: "c" = s-add + fused evac-add (DVE/Pool), "sigma" = inject l,r
# via I-weight matmuls on PE + plain ScalarE copy evac.  Tuned by sweeping.
SIGMA_ITEMS = {(0, 0), (0, 1), (0, 2), (1, 0), (1, 2), (2, 2), (3, 2),
               (4, 2), (6, 0), (6, 1), (6, 2)}
EVAC_V_ITEMS = {(5, 2)}
ENG = {
    "dX": lambda it, i: "v",
    "pX": lambda it, i: "v",
    "dZ": lambda it, i: "g",
    "pZ": lambda it, i: "g",
    "pY": lambda it, i: "v",
    "s":  lambda it, i: "v",
    "evac": lambda it, i: "v" if (it, i) in EVAC_V_ITEMS else "g",
    "route": lambda it, i: "sigma" if (it, i) in SIGMA_ITEMS else "c",
    "odma": lambda it, i: "s",
}


def build_nc(xs: int = XS) -> bass.Bass:
    xh = xs + 2
    nc = bacc.Bacc(None)

    in_h = [nc.declare_dram_parameter(f"ch{i}", [Y, xh, 2, ZP], F16,
                                      isOutput=False) for i in range(D)]
    w_h = nc.declare_dram_parameter("wmats", [Y, 4 * Y], F16, isOutput=False)
    out_h = nc.declare_dram_parameter("out", [D, Y, xs, Z], F16, isOutput=True)

    S_H = ZP           # pair-half stride (elements)
    S_X = 2 * ZP       # x-plane stride (elements)

    if xs == XS:
        chunks = CHUNKS
    else:
        chunks = [min(8, xs)] * max(1, xs // min(8, xs))
    cuts = [0]
    off = 0
    for kb in chunks:
        off += kb
        cuts.append(min(xh, off + 2))
    while len(cuts) >= 2 and cuts[-2] == cuts[-1]:
        cuts.pop()
    if cuts[-1] != xh:
        cuts.append(xh)

    def eng(kind, it, i):
        return nc.vector if ENG[kind](it, i) == "v" else nc.gpsimd

    def pair_shift_x(t, x0, kb):
        base = t[:, :, :, :]
        p = base.ap[0][0]
        off = base.offset + x0 * S_X + 1
        in0 = AP(base.tensor, off + 2 * S_X,
                 [[p, Y], [S_H - 2 * S_X, 2], [S_X, kb], [1, Z]])
        in1 = AP(base.tensor, off,
                 [[p, Y], [S_H + 2 * S_X, 2], [S_X, kb], [1, Z]])
        return in0, in1

    def pair_shift_z(t, x0, kb):
        base = t[:, :, :, :]
        p = base.ap[0][0]
        off = base.offset + (x0 + 1) * S_X
        in0 = AP(base.tensor, off + 2,
                 [[p, Y], [S_H - 2, 2], [S_X, kb], [1, Z]])
        in1 = AP(base.tensor, off,
                 [[p, Y], [S_H + 2, 2], [S_X, kb], [1, Z]])
        return in0, in1

    def pair_swap(t, x0, kb):
        base = t[:, :, :, :]
        p = base.ap[0][0]
        off = base.offset + S_H + (x0 + 1) * S_X + 1
        return AP(base.tensor, off,
                  [[p, Y], [-S_H, 2], [S_X, kb], [1, Z]])

    with tile.TileContext(nc) as tc:
        with (
            tc.tile_pool(name="inp", bufs=1) as inp,
            tc.tile_pool(name="wp", bufs=1) as wp,
            tc.tile_pool(name="dpool", bufs=3) as dpool,
            tc.tile_pool(name="ppool", bufs=3) as ppool,
            tc.tile_pool(name="ypool", bufs=3) as ypool,
            tc.tile_pool(name="spool", bufs=3) as spool,
            tc.tile_pool(name="opool", bufs=8) as opool,
            tc.tile_pool(name="psum_dy", bufs=2, space="PSUM") as psum_dy,
            tc.tile_pool(name="psum_acc", bufs=4, space="PSUM") as psum_acc,
        ):
            wt = wp.tile([Y, 4 * Y], F16, name="wt")
            dyT = wt[:, 0:Y]            # 0.25*DyT
            ndyT = wt[:, Y:2 * Y]       # -0.25*DyT
            eyeT = wt[:, 2 * Y:3 * Y]   # I
            qeyeT = wt[:, 3 * Y:4 * Y]  # 0.25*I

            ct = []
            for i in range(D):
                t = inp.tile([Y, xh, 2, ZP], F16, name=f"ch{i}", tag=f"ch{i}")
                ct.append(t)
            nc.sync.dma_start(out=wt[:, :], in_=w_h[:, :])
            for a, b2 in zip(cuts, cuts[1:]):
                for i in range(D):
                    nc.sync.dma_start(out=ct[i][:, a:b2, :, :],
                                      in_=in_h[i][:, a:b2, :, :])

            zc = slice(1, 1 + Z)

            # Prime PE's vector clock against every input DMA chunk.
            scratch = psum_acc.tile([8, 8], F32, name="scratch", tag="acc")
            for a in cuts[:-1]:
                for t in ct:
                    nc.tensor.matmul(scratch[:, 0:1], wt[:, 0:8],
                                     t[:, a:a + 1, 0:1, 0:1],
                                     start=True, stop=True)

            items = []
            off = 0
            for kb in chunks:
                items.append((off, kb))
                off += kb
            assert off == xs

            def stage_a(it, i):
                x0, kb = items[it]
                dX = dpool.tile([Y, 2, kb, Z], F16, name="dX", tag="dX")
                i0, i1 = pair_shift_x(ct[i], x0, kb)
                eng("dX", it, i).tensor_sub(out=dX[:, :, :, :], in0=i0, in1=i1)
                pX = ppool.tile([Y, 2, kb, Z], F16, name="pX", tag="pX")
                eng("pX", it, i).tensor_mul(out=pX[:, :, :, :],
                                            in0=dX[:, :, :, :],
                                            in1=pair_swap(ct[0], x0, kb))
                dZ = dpool.tile([Y, 2, kb, Z], F16, name="dZ", tag="dZ")
                j0, j1 = pair_shift_z(ct[i], x0, kb)
                eng("dZ", it, i).tensor_sub(out=dZ[:, :, :, :], in0=j0, in1=j1)
                pZ = ppool.tile([Y, 2, kb, Z], F16, name="pZ", tag="pZ")
                eng("pZ", it, i).tensor_mul(out=pZ[:, :, :, :],
                                            in0=dZ[:, :, :, :],
                                            in1=pair_swap(ct[2], x0, kb))
                if ENG["route"](it, i) not in ("sigma", "r8"):
                    s = spool.tile([Y, kb, Z], F16, name="s", tag="s")
                    eng("s", it, i).tensor_add(
                        out=s[:, :, :],
                        in0=ct[i][:, 1 + x0:1 + x0 + kb, 0, zc],
                        in1=ct[i][:, 1 + x0:1 + x0 + kb, 1, zc])
                else:
                    s = None
                return pX, pZ, s

            def stage_y(it, i):
                x0, kb = items[it]
                dyS = ypool.tile([Y, 2, kb, Z], F16, name="dyS", tag="dyS")
                for ho, kx in acc_chunks(kb):
                    x0h = x0 + ho
                    hs = slice(1 + x0h, 1 + x0h + kx)
                    dyP = psum_dy.tile([Y, 2, kx, Z], F32,
                                       name="dyP", tag="dyP")
                    nc.tensor.matmul(dyP[:, 0, :, :], dyT,
                                     ct[i][:, hs, 0, zc],
                                     start=True, stop=True)
                    nc.tensor.matmul(dyP[:, 1, :, :], ndyT,
                                     ct[i][:, hs, 1, zc],
                                     start=True, stop=True)
                    nc.scalar.copy(out=dyS[:, :, ho:ho + kx, :],
                                   in_=dyP[:, :, :, :])
                pY = ppool.tile([Y, 2, kb, Z], F16, name="pY", tag="pY")
                eng("pY", it, i).tensor_mul(out=pY[:, :, :, :],
                                            in0=dyS[:, :, :, :],
                                            in1=pair_swap(ct[1], x0, kb))
                return pY

            def acc_chunks(kb):
                out = []
                o = 0
                while o < kb:
                    k = min(KX, kb - o)
                    out.append((o, k))
                    o += k
                return out

            def stage_b(it, i, h, pX, pZ, s, pY, ot):
                x0, kb = items[it]
                ho, kx = acc_chunks(kb)[h]
                ph = slice(ho, ho + kx)
                sig = ENG["route"](it, i) in ("sigma", "r8")
                hs = slice(1 + x0 + ho, 1 + x0 + ho + kx)
                acc = psum_acc.tile([Y, kx, Z], F32, name="acc", tag="acc")
                nc.tensor.matmul(acc[:, :, :], qeyeT, pX[:, 0, ph, :],
                                 start=True, stop=False)
                nc.tensor.matmul(acc[:, :, :], qeyeT, pX[:, 1, ph, :],
                                 start=False, stop=False)
                nc.tensor.matmul(acc[:, :, :], qeyeT, pZ[:, 0, ph, :],
                                 start=False, stop=False)
                nc.tensor.matmul(acc[:, :, :], qeyeT, pZ[:, 1, ph, :],
                                 start=False, stop=False)
                nc.tensor.matmul(acc[:, :, :], eyeT, pY[:, 0, ph, :],
                                 start=False, stop=False)
                if sig:
                    nc.tensor.matmul(acc[:, :, :], eyeT, pY[:, 1, ph, :],
                                     start=False, stop=False)
                    nc.tensor.matmul(acc[:, :, :], eyeT,
                                     ct[i][:, hs, 0, zc],
                                     start=False, stop=False)
                    nc.tensor.matmul(acc[:, :, :], eyeT,
                                     ct[i][:, hs, 1, zc],
                                     start=False, stop=True)
                    nc.scalar.copy(out=ot[:, ph, :], in_=acc[:, :, :])
                else:
                    nc.tensor.matmul(acc[:, :, :], eyeT, pY[:, 1, ph, :],
                                     start=False, stop=True)
                    # HW: GPSIMD cannot touch PSUM -> Act evacuates, then
                    # a cheap fp16 2x add folds in the linear term.
                    nc.scalar.copy(out=ot[:, ph, :], in_=acc[:, :, :])
                    eng("evac", it, i).tensor_add(out=ot[:, ph, :],
                                                  in0=ot[:, ph, :],
                                                  in1=s[:, ph, :])

            def nkx(it):
                x0, kb = items[it]
                return len(acc_chunks(kb))

            work = [(it, i) for it in range(len(items)) for i in range(D)]

            def run_b(entry):
                (pit, pi), (ppX, ppZ, ps), pys, pot = entry
                for h in range(nkx(pit)):
                    stage_b(pit, pi, h, ppX, ppZ, ps, pys, pot)
                px0, pkb = items[pit]
                oeng = nc.sync if ENG["odma"](pit, pi) == "s" else nc.scalar
                oeng.dma_start(out=out_h[pi, :, px0:px0 + pkb, :],
                               in_=pot[:, :, :])

            pending = []
            for it, i in work:
                cur_a = stage_a(it, i)
                cur_y = stage_y(it, i)
                x0, kb = items[it]
                ot = opool.tile([Y, kb, Z], F16, name="ot", tag="ot")
                pending.append(((it, i), cur_a, cur_y, ot))
                if len(pending) > DEPTH:
                    run_b(pending.pop(0))
            for entry in pending:
                run_b(entry)

    if not nc.is_finalized():
        nc.finalize()
    return nc


def _host_shard(l_b: np.ndarray, r_b: np.ndarray, xs: int) -> list[dict]:
    """(D, X, Y, Z) f32 pair -> per-slab dicts of (Y, 2, xs+2, ZP) fp16."""
    out = []
    for s0 in range(X // xs):
        idx = (np.arange(-1, xs + 1) + s0 * xs) % X
        m = {}
        for i in range(D):
            sl = np.stack([l_b[i][idx], r_b[i][idx]], axis=0)  # (2,xh,Y,Z)
            sl = np.transpose(sl, (2, 1, 0, 3))                # (Y,xh,2,Z)
            sl = np.concatenate([sl[..., Z - 1:Z], sl, sl[..., 0:1]], axis=-1)
            m[f"ch{i}"] = np.ascontiguousarray(sl.astype(np.float16))
        out.append(m)
    return out


def kernel(left: np.ndarray, right: np.ndarray) -> np.ndarray:
    left = np.asarray(left)
    right = np.asarray(right)
    assert left.shape == (B, D, X, Y, Z), left.shape

    wmats = _make_wmats()
    slabs = X // XS  # 4

    shards = [_host_shard(np.asarray(left[b], dtype=np.float32),
                          np.asarray(right[b], dtype=np.float32), XS)
              for b in range(B)]

    maps = []
    for core in range(NCORES):
        b, s0 = divmod(core, slabs)
        m = dict(shards[b][s0])
        m["wmats"] = wmats
        maps.append(m)

    nc = build_nc(XS)
    res = run_bass_kernel_spmd(nc, maps, core_ids=list(range(NCORES)))

    out = np.empty((B, D, X, Y, Z), dtype=np.float32)
    for core in range(NCORES):
        b, s0 = divmod(core, slabs)
        o = res.results[core]["out"].astype(np.float32)   # (D, Y, XS, Z)
        out[b, :, s0 * XS:(s0 + 1) * XS, :, :] = np.transpose(o, (0, 2, 1, 3))
    return out


# ---------------------------------------------------------------------------
# numpy reference of the same math (for probing without jax)
def _np_ref(left: np.ndarray, right: np.ndarray) -> np.ndarray:
    l = np.moveaxis(left, 1, -1).astype(np.float64)
    r = np.moveaxis(right, 1, -1).astype(np.float64)

    def jac(v):
        cols = []
        for j in range(3):
            ax = 1 + j
            g = (np.roll(v, -1, axis=ax) - np.roll(v, 1, axis=ax)) * 0.5
            cols.append(g)
        return np.stack(cols, axis=-1)

    jx, jy = jac(l), jac(r)
    br = np.einsum("bxyzij,bxyzj->bxyzi", jx, r) - np.einsum(
        "bxyzij,bxyzj->bxyzi", jy, l)
    z = l + r + 0.5 * br
    return np.moveaxis(z, -1, 1).astype(np.float32)
